# revision 4
# baseline (speedup 1.0000x reference)
"""Multi-head causal attention (LLaMA RoPE) on 8 Trainium2 cores, v6.

Sharding: core c -> (batch b = c//2, head-half hh = c%2, i.e. heads
[8*hh, 8*hh+8)). Each core projects Q/K/V for its 8 heads over all 1024
rows (no duplicated projection work), runs causal attention for those
heads over the full sequence, then pairs (2b, 2b+1) exchange attention
outputs with staggered pair-wise AllGathers so each core can apply the
full output projection for the 512 rows it owns (core 2b: rows 0-511,
core 2b+1: rows 512-1023). Per-core program is identical; per-core
behavior comes from input data + partition_id (drives the dynamic DMA
offsets selecting "my rows" / "partner slot").

Numerics: all matmul operands bf16 (PE full rate at any tile size),
accumulation fp32 in PSUM, biases/normalization fp32. Softmax skips
max-subtraction (logits are O(5)); denominator comes from a ones-matmul
accumulated alongside PV. Causality: k-tiles strictly above the
diagonal are skipped; diagonal 128x128 blocks are zeroed after exp with
a multiplicative 0/1 mask on the DVE (derived from the mask input).

Attention processes full q-strips [kt*128, 1024) per k-tile with
2-bank (4KB/partition) PSUM tiles: one exp per k-tile keeps the Act
engine under the PE's work, and a depth-3 software pipeline (PV and
denominator matmuls for k-tile t issue under ST of k-tile t+3) covers
both the exp latency and the per-head normalize drain. The output
projection's weights and attention-exchange chunks stream during
attention into buffers whose previous tenants (x, projection weights)
are dead, so the O-projection starts immediately at attention end with
all but the last gather's chunks resident.
"""

import math
import sys

import numpy as np

sys.path.insert(0, "/opt/trn_rl_repo")

from ml_dtypes import bfloat16

B, S, DIM, H = 4, 1024, 2048, 16
HD = DIM // H  # 128
HPC = 8  # heads per core
KC = DIM // 128  # 16 contraction chunks
NKT = S // 128  # 8 k tiles
NQ = S // 2  # 512 rows owned per core
SCALE = 1.0 / math.sqrt(HD)
N_CORES = 8
GROUPS = [[0, 1], [2, 3], [4, 5], [6, 7]]

_cache = {}


def _build_nc():
    import concourse.bass as bass
    import concourse.mybir as mybir
    import concourse.tile as tile
    from concourse import bacc

    BF16 = mybir.dt.bfloat16
    F32 = mybir.dt.float32
    ds = bass.ds
    Act = mybir.ActivationFunctionType

    nc = bacc.Bacc("TRN2", target_bir_lowering=False, debug=False,
                   num_devices=N_CORES)

    x_in = nc.dram_tensor("x_pre", [128, KC, S], BF16, kind="ExternalInput")
    wq_in = nc.dram_tensor("wq_pre", [HPC, 128, KC, 128], BF16, kind="ExternalInput")
    wk_in = nc.dram_tensor("wk_pre", [HPC, 128, KC, 128], BF16, kind="ExternalInput")
    wv_in = nc.dram_tensor("wv_pre", [128, KC, 2, 512], BF16, kind="ExternalInput")
    wo_in = nc.dram_tensor("wo_pre", [128, KC, 4, 512], BF16, kind="ExternalInput")
    bq_in = nc.dram_tensor("bq_p", [128, HPC, 1], F32, kind="ExternalInput")
    bk_in = nc.dram_tensor("bk_p", [128, HPC, 1], F32, kind="ExternalInput")
    bv_in = nc.dram_tensor("bv_p", [128, 2, 512], F32, kind="ExternalInput")
    csk_in = nc.dram_tensor("csk2", [128, S], BF16, kind="ExternalInput")
    ssk_in = nc.dram_tensor("ssk2", [128, S], BF16, kind="ExternalInput")
    tri_in = nc.dram_tensor("tri01", [128, 128], BF16, kind="ExternalInput")
    ones_in = nc.dram_tensor("ones128", [128, 128], BF16, kind="ExternalInput")
    y_out = nc.dram_tensor("y", [NQ, DIM], F32, kind="ExternalOutput")

    with tile.TileContext(nc) as tc:
        with (
            tc.tile_pool(name="consts", bufs=1) as consts,
            tc.tile_pool(name="xpool", bufs=1) as xpool,
            tc.tile_pool(name="qkv", bufs=1) as qkv,
            tc.tile_pool(name="wqk", bufs=4) as wqk,
            tc.tile_pool(name="combp", bufs=4) as combp,
            tc.tile_pool(name="wbig", bufs=2) as wbig,
            tc.tile_pool(name="rope", bufs=3) as rope,
            tc.tile_pool(name="ptp", bufs=5) as ptp,
            tc.tile_pool(name="dram", bufs=1, space="DRAM") as dram,
            tc.tile_pool(name="psB", bufs=2, space="PSUM") as psB,
            tc.tile_pool(name="psL1", bufs=1, space="PSUM") as psL1,
            tc.tile_pool(name="psO1", bufs=1, space="PSUM") as psO1,
        ):
            # ---- DMAs in consumption order: first Q/K weights + x ----
            wq_sbs = {}
            wk_sbs = {}

            def fetch_w(which, h, nsplit=1):
                t = wqk.tile([128, KC, 128], BF16, tag="w",
                             name=f"w{which}_sb")
                src = (wq_in if which == "q" else wk_in)[h]
                step = KC // nsplit
                for j in range(0, KC, step):
                    nc.sync.dma_start(t[:, j:j + step, :], src[:, j:j + step, :])
                (wq_sbs if which == "q" else wk_sbs)[h] = t

            # x streams on the Act engine's DMA queue, weights on sync's,
            # so the two overlap from t=0. First tiles split fine so the
            # first matmul's semaphore wait covers minimal data.
            x_sb = xpool.tile([128, KC, S], BF16, tag="x", name="x_sb")
            fetch_w("q", 0, nsplit=4)
            nc.scalar.dma_start(x_sb[:, 0:1, :], x_in[:, 0:1, :])
            nc.scalar.dma_start(x_sb[:, 1:2, :], x_in[:, 1:2, :])
            fetch_w("k", 0, nsplit=2)
            nc.scalar.dma_start(x_sb[:, 2:4, :], x_in[:, 2:4, :])
            bq_sb = consts.tile([128, HPC, 1], F32, tag="bq")
            bk_sb = consts.tile([128, HPC, 1], F32, tag="bk")
            nc.sync.dma_start(bq_sb[:], bq_in[:])
            nc.sync.dma_start(bk_sb[:], bk_in[:])
            fetch_w("q", 1)
            fetch_w("k", 1)
            for g in range(2, 8):
                nc.scalar.dma_start(x_sb[:, g * 2:g * 2 + 2, :],
                                    x_in[:, g * 2:g * 2 + 2, :])
            csk_sb = consts.tile([128, S], BF16, tag="cs", name="csk_sb")
            ssk_sb = consts.tile([128, S], BF16, tag="ss", name="ssk_sb")
            nc.sync.dma_start(csk_sb[:], csk_in[:])
            nc.sync.dma_start(ssk_sb[:], ssk_in[:])
            ones_sb = consts.tile([128, 128], BF16)
            nc.sync.dma_start(ones_sb[:], ones_in[:])
            tri_sb = consts.tile([128, 128], BF16)
            nc.sync.dma_start(tri_sb[:], tri_in[:])
            bv_sb = consts.tile([128, 2, 512], F32, tag="bv")
            nc.sync.dma_start(bv_sb[:], bv_in[:])

            q_sb = qkv.tile([128, HPC, S], BF16, tag="q", name="q_sb")
            k_sb = qkv.tile([128, HPC, S], BF16, tag="k", name="k_sb")
            v_sb = qkv.tile([128, NKT, 2, 512], BF16, tag="v", name="v_sb")
            ot_sb = qkv.tile([128, HPC, S], BF16, tag="ot", name="ot_sb")

            def rope_block(pm, b_ap, dst_ap):
                """dst = rope(pm + bias); pm [128, S] psum."""
                tmp_t = rope.tile([128, S], BF16, tag="tmp", name="tmp_t")
                nc.scalar.activation(tmp_t[:], pm, Act.Identity, bias=b_ap)
                tswap_t = rope.tile([128, S], BF16, tag="tswap", name="tswap_t")
                nc.scalar.dma_start(tswap_t[0:64, :], tmp_t[64:128, :])
                nc.scalar.dma_start(tswap_t[64:128, :], tmp_t[0:64, :])
                nc.vector.tensor_mul(tmp_t[:], tmp_t[:], csk_sb[:])
                nc.vector.tensor_mul(tswap_t[:], tswap_t[:], ssk_sb[:])
                nc.vector.tensor_add(dst_ap, tmp_t[:], tswap_t[:])

            # ---- Q/K projections + RoPE (weights double-prefetched) ----
            HALVES = [slice(0, 512), slice(512, 1024)]
            for h in range(HPC):
                wq_sb = wq_sbs[h]
                pm = psB.tile([128, S], F32, tag="big", name="pmq")
                for kc in range(KC):
                    for hs in HALVES:
                        nc.tensor.matmul(pm[:, hs], wq_sb[:, kc, :],
                                         x_sb[:, kc, hs],
                                         start=(kc == 0), stop=(kc == KC - 1))
                rope_block(pm[:], bq_sb[:, h, :], q_sb[:, h, :])
                if h + 2 < HPC:
                    fetch_w("q", h + 2)
                wk_sb = wk_sbs[h]
                pm = psB.tile([128, S], F32, tag="big", name="pmk")
                for kc in range(KC):
                    for hs in HALVES:
                        nc.tensor.matmul(pm[:, hs], wk_sb[:, kc, :],
                                         x_sb[:, kc, hs],
                                         start=(kc == 0), stop=(kc == KC - 1))
                rope_block(pm[:], bk_sb[:, h, :], k_sb[:, h, :])
                if h + 2 < HPC:
                    fetch_w("k", h + 2)

            # ---- V projection (transposed: rows on partitions) ----
            for eb in range(2):
                wv_sb = wbig.tile([128, KC, 512], BF16, tag="wv", name="wv_sb")
                nc.sync.dma_start(wv_sb[:], wv_in[:, :, eb, :])
                for rc in range(NKT):
                    rcols = slice(rc * 128, rc * 128 + 128)
                    pm = psB.tile([128, S], F32, tag="big", name="pmv")
                    for kc in range(KC):
                        nc.tensor.matmul(pm[:, 0:512], x_sb[:, kc, rcols],
                                         wv_sb[:, kc, :],
                                         start=(kc == 0), stop=(kc == KC - 1))
                    nc.vector.tensor_add(v_sb[:, rc, eb, :], pm[:, 0:512],
                                         bv_sb[:, eb, :])

            # ---- pair-exchange plumbing ----
            pid = nc.sync.partition_id()
            myrh = pid % 2
            prh = (pid + 1) % 2
            # staggered gathers: heads 0-3, 4-5, 6, 7 (tail kept tiny so
            # the final gather lands before the O-projection needs it)
            AG_HEADS = [(0, 4), (4, 2), (6, 1), (7, 1)]
            cc_in = [dram.tile([128, nh, 512], BF16, name=f"cc_in{g}")
                     for g, (h0, nh) in enumerate(AG_HEADS)]
            cc_out = [dram.tile([2, 128, nh, 512], BF16, name=f"cc_out{g}")
                      for g, (h0, nh) in enumerate(AG_HEADS)]
            # comb quarters: O-projection d_in chunks, local-first order
            # (0-7 = my heads for my rows, 8-15 = partner heads; wo_pre is
            # packed per-core with the matching chunk order). Quarters so
            # each lands as soon as its heads/gather finish.
            combq = [combp.tile([128, 4, 512], BF16, tag="cq",
                                name=f"combq{j}") for j in range(4)]
            wo23 = []

            # ---- attention: full q-strips, depth-3 pipeline ----
            def vtile(h, kt):
                return v_sb[:, kt, h // 4, (h % 4) * 128:(h % 4) * 128 + 128]

            def segs(kt):
                lo = kt * 128
                if lo < 512:
                    return [slice(lo, 512), slice(512, 1024)]
                return [slice(lo, 1024)]

            head_acc = {}
            pending = []
            wo_store = {}

            def post_head(h):
                if h in (3, 5, 6, 7):
                    g = {3: 0, 5: 1, 6: 2, 7: 3}[h]
                    h0, nh = AG_HEADS[g]
                    nc.sync.dma_start(
                        cc_in[g][:],
                        ot_sb[:, h0:h0 + nh, ds(prh * 512, 512)])
                    nc.gpsimd.collective_compute(
                        "AllGather",
                        mybir.AluOpType.bypass,
                        replica_groups=GROUPS,
                        ins=[cc_in[g][:].opt()],
                        outs=[cc_out[g][:].opt()],
                    )
                    # partner slot -> comb chunks 8+ (static dst)
                    coff = HPC + sum(n for _, n in AG_HEADS[:g])
                    nc.sync.dma_start(
                        combq[coff // 4][:, coff % 4:coff % 4 + nh, :],
                        cc_out[g][ds(prh, 1)][0])
                # stream wo during attention: ob 0-1 into the dead x_sb
                # buffer, ob 2-3 into the wv pool
                if h == 3:
                    t = xpool.tile([128, KC, 2, 512], BF16, tag="x",
                                   name="wo01_sb")
                    nc.sync.dma_start(t[:], wo_in[:, :, 0:2, :])
                    wo_store["01"] = t
                elif h in (5, 6):
                    t = wbig.tile([128, KC, 512], BF16, tag="wv",
                                  name="wo_sb")
                    nc.sync.dma_start(t[:], wo_in[:, :, h - 3, :])
                    wo23.append(t)

            def emit_lo(h, kt, pt):
                l_ps, o_ps = head_acc[h]
                for sg in segs(kt):
                    nc.tensor.matmul(l_ps[:, sg], ones_sb[:], pt[:, sg],
                                     start=(kt == 0), stop=(kt == NKT - 1))
                for sg in segs(kt):
                    nc.tensor.matmul(o_ps[:, sg], vtile(h, kt), pt[:, sg],
                                     start=(kt == 0), stop=(kt == NKT - 1))
                if kt == NKT - 1:
                    finish_head(h)

            def finish_head(h):
                l_ps, o_ps = head_acc.pop(h)
                rl = consts.tile([128, S], F32, tag="rl", name="rl")
                nc.vector.reciprocal_approx_fast(rl[:], l_ps[:])
                nc.vector.tensor_mul(ot_sb[:, h, :], o_ps[:], rl[:])
                # my-rows slice of this head -> comb chunk h (static dst)
                nc.sync.dma_start(combq[h // 4][:, h % 4, :],
                                  ot_sb[:, h, ds(myrh * 512, 512)])
                post_head(h)

            for h in range(HPC):
                head_acc[h] = (psL1.tile([128, S], F32, tag="l", name="l_ps"),
                               psO1.tile([128, S], F32, tag="o", name="o_ps"))
                for kt in range(NKT):
                    lo = kt * 128
                    st = psB.tile([128, S], F32, tag="big", name="st_ps")
                    for sg in segs(kt):
                        nc.tensor.matmul(
                            st[:, sg], k_sb[:, h, kt * 128:kt * 128 + 128],
                            q_sb[:, h, sg], start=True, stop=True)
                    pt = ptp.tile([128, S], BF16, tag="pt", name="pt")
                    nc.scalar.activation(pt[:, lo:1024], st[:, lo:1024],
                                         Act.Exp, scale=SCALE)
                    # diagonal block: zero out q < k after exp
                    nc.vector.tensor_mul(pt[:, lo:lo + 128],
                                         pt[:, lo:lo + 128], tri_sb[:])
                    pending.append((h, kt, pt))
                    if len(pending) > 3:
                        emit_lo(*pending.pop(0))
            while pending:
                emit_lo(*pending.pop(0))
            wo01_sb = wo_store["01"]

            # ---- output projection for my 512 rows ----
            # process ob-pairs with all 8 PSUM banks open and the final
            # gather's chunk (15) deferred to the pair's end, so ~25us of
            # matmuls cover the last AllGather's latency + pair skew.
            def omm(pms2, ob, i, qc):
                qsl = slice(qc * 128, qc * 128 + 128)
                osl = slice((qc % 2) * 512, (qc % 2) * 512 + 512)
                nc.tensor.matmul(pms2[qc // 2][:, osl],
                                 combq[i // 4][:, i % 4, qsl],
                                 (wo01_sb[:, i, ob, :] if ob < 2
                                  else wo23[ob - 2][:, i, :]),
                                 start=(i == 0), stop=(i == KC - 1))

            def drain(pms2, ob, qc):
                qsl = slice(qc * 128, qc * 128 + 128)
                osl = slice((qc % 2) * 512, (qc % 2) * 512 + 512)
                y_sb = rope.tile([128, 512], F32, tag="tswap", name="y_sb")
                if qc % 2 == 0:
                    nc.vector.tensor_copy(y_sb[:], pms2[qc // 2][:, osl])
                    nc.sync.dma_start(y_out[qsl, ob * 512:ob * 512 + 512],
                                      y_sb[:])
                else:
                    nc.scalar.activation(y_sb[:], pms2[qc // 2][:, osl],
                                         Act.Identity)
                    nc.scalar.dma_start(
                        y_out[qsl, ob * 512:ob * 512 + 512], y_sb[:])

            for pair in range(2):
                obs = (2 * pair, 2 * pair + 1)
                pmss = {
                    obs[0]: [psB.tile([128, S], F32, tag="big", name="pmoA"),
                             psB.tile([128, S], F32, tag="big", name="pmoB")],
                    obs[1]: [psL1.tile([128, S], F32, tag="l", name="pmoC"),
                             psO1.tile([128, S], F32, tag="o", name="pmoD")],
                }
                for i in range(KC - 1):
                    for ob in obs:
                        for qc in range(4):
                            omm(pmss[ob], ob, i, qc)
                for ob in obs:
                    for qc in range(4):
                        omm(pmss[ob], ob, KC - 1, qc)
                        drain(pmss[ob], ob, qc)
    nc.compile()
    return nc


def _get_nc():
    if "nc" not in _cache:
        _cache["nc"] = _build_nc()
    return _cache["nc"]


def _evenodd(a):
    # permute within-head dim: even indices first, then odd (axis 0)
    return np.concatenate([a[0::2], a[1::2]], axis=0)


def kernel(**inputs):
    from concourse.bass_utils import run_bass_kernel_spmd

    trace = bool(inputs.pop("_trace", False))
    x = np.asarray(inputs["x"], np.float32)
    freqs_cos = np.asarray(inputs["freqs_cos"], np.float32)
    freqs_sin = np.asarray(inputs["freqs_sin"], np.float32)
    mask = np.asarray(inputs["mask"], np.float32)
    wq = np.asarray(inputs["wq"], np.float32)
    bq = np.asarray(inputs["bq"], np.float32)
    wk = np.asarray(inputs["wk"], np.float32)
    bk = np.asarray(inputs["bk"], np.float32)
    wv = np.asarray(inputs["wv"], np.float32)
    bv = np.asarray(inputs["bv"], np.float32)
    wo = np.asarray(inputs["wo"], np.float32)
    bo = np.asarray(inputs["bo"], np.float32)

    cosT = freqs_cos.T
    sinT = freqs_sin.T
    csk2 = np.vstack([cosT, cosT]).astype(bfloat16)
    ssk2 = np.vstack([-sinT, sinT]).astype(bfloat16)

    m2 = mask[0, 0]  # [S(q), S(k)] additive
    # multiplicative 0/1 within-block causal mask, [k, q] layout
    tri01 = (m2[:128, :128].T == 0.0).astype(np.float32).astype(bfloat16)
    ones128 = np.ones((128, 128), np.float32).astype(bfloat16)

    def pack_thin(w_half):
        out = np.empty((HPC, 128, KC, 128), bfloat16)
        for h in range(HPC):
            rows = _evenodd(w_half[h * 128:(h + 1) * 128])  # [128, 2048]
            out[h] = rows.T.reshape(KC, 128, 128).transpose(1, 0, 2).astype(bfloat16)
        return out

    halves = []
    for hh in range(2):
        sl = slice(hh * 1024, hh * 1024 + 1024)
        wq_pre = pack_thin(wq[sl])
        wk_pre = pack_thin(wk[sl])
        bq_p = np.empty((128, HPC, 1), np.float32)
        bk_p = np.empty((128, HPC, 1), np.float32)
        for h in range(HPC):
            bq_p[:, h, 0] = _evenodd(bq[hh * 1024 + h * 128:hh * 1024 + (h + 1) * 128])
            bk_p[:, h, 0] = _evenodd(bk[hh * 1024 + h * 128:hh * 1024 + (h + 1) * 128])
        wv_pre = np.ascontiguousarray(
            wv[sl].T.reshape(KC, 128, 2, 512).transpose(1, 0, 2, 3)
        ).astype(bfloat16)
        # wo d_in chunks rolled so local head chunks come first (matches
        # comb's local-first layout in the kernel)
        order = list(range(hh * 8, hh * 8 + 8)) + \
            list(range((1 - hh) * 8, (1 - hh) * 8 + 8))
        woT = wo.T.reshape(KC, 128, 4, 512)
        wo_pre = np.ascontiguousarray(
            woT[order].transpose(1, 0, 2, 3)
        ).astype(bfloat16)
        halves.append((wq_pre, wk_pre, bq_p, bk_p, wv_pre, wo_pre))

    common = {
        "csk2": csk2, "ssk2": ssk2,
        "tri01": tri01, "ones128": ones128,
    }
    in_maps = []
    for c in range(N_CORES):
        b, hh = c // 2, c % 2
        wq_pre, wk_pre, bq_p, bk_p, wv_pre, wo_pre = halves[hh]
        x_pre = np.ascontiguousarray(
            x[b].T.reshape(KC, 128, S).transpose(1, 0, 2)
        ).astype(bfloat16)
        bv_p = np.ascontiguousarray(
            np.broadcast_to(bv[hh * 1024:hh * 1024 + 1024].reshape(1, 2, 512),
                            (128, 2, 512))
        )
        in_maps.append({
            **common,
            "x_pre": x_pre,
            "wq_pre": wq_pre, "wk_pre": wk_pre,
            "bq_p": bq_p, "bk_p": bk_p,
            "wv_pre": wv_pre, "bv_p": bv_p, "wo_pre": wo_pre,
        })

    nc = _get_nc()
    kwargs = {}
    if trace:
        kwargs = {"trace": True, "trace_cores": list(range(N_CORES))}
    res = run_bass_kernel_spmd(nc, in_maps, core_ids=list(range(N_CORES)), **kwargs)
    _cache["last_result"] = res

    out = np.empty((B, S, DIM), np.float32)
    for c in range(N_CORES):
        b, hh = c // 2, c % 2
        out[b, hh * NQ:hh * NQ + NQ] = res.results[c]["y"] + bo[None, :]
    return out


# revision 5
# speedup vs baseline: 1.0867x; 1.0867x over previous
"""Multi-head causal attention (LLaMA RoPE) on 8 Trainium2 cores, v6.

Sharding: core c -> (batch b = c//2, head-half hh = c%2, i.e. heads
[8*hh, 8*hh+8)). Each core projects Q/K/V for its 8 heads over all 1024
rows (no duplicated projection work), runs causal attention for those
heads over the full sequence, then pairs (2b, 2b+1) exchange attention
outputs with staggered pair-wise AllGathers so each core can apply the
full output projection for the 512 rows it owns (core 2b: rows 0-511,
core 2b+1: rows 512-1023). Per-core program is identical; per-core
behavior comes from input data + partition_id (drives the dynamic DMA
offsets selecting "my rows" / "partner slot").

Numerics: all matmul operands bf16 (PE full rate at any tile size),
accumulation fp32 in PSUM, biases/normalization fp32. Softmax skips
max-subtraction (logits are O(5)); denominator comes from a ones-matmul
accumulated alongside PV. Causality: k-tiles strictly above the
diagonal are skipped; diagonal 128x128 blocks are zeroed after exp with
a multiplicative 0/1 mask on the DVE (derived from the mask input).

Attention processes full q-strips [kt*128, 1024) per k-tile with
2-bank (4KB/partition) PSUM tiles: one exp per k-tile keeps the Act
engine under the PE's work, and a depth-3 software pipeline (PV and
denominator matmuls for k-tile t issue under ST of k-tile t+3) covers
both the exp latency and the per-head normalize drain. The output
projection's weights and attention-exchange chunks stream during
attention into buffers whose previous tenants (x, projection weights)
are dead, so the O-projection starts immediately at attention end with
all but the last gather's chunks resident.
"""

import math
import sys

import numpy as np

sys.path.insert(0, "/opt/trn_rl_repo")

from ml_dtypes import bfloat16

B, S, DIM, H = 4, 1024, 2048, 16
HD = DIM // H  # 128
HPC = 8  # heads per core
KC = DIM // 128  # 16 contraction chunks
NKT = S // 128  # 8 k tiles
NQ = S // 2  # 512 rows owned per core
SCALE = 1.0 / math.sqrt(HD)
N_CORES = 8
GROUPS = [[0, 1], [2, 3], [4, 5], [6, 7]]

_cache = {}


def _build_nc():
    import concourse.bass as bass
    import concourse.mybir as mybir
    import concourse.tile as tile
    from concourse import bacc

    BF16 = mybir.dt.bfloat16
    F32 = mybir.dt.float32
    ds = bass.ds
    Act = mybir.ActivationFunctionType

    nc = bacc.Bacc("TRN2", target_bir_lowering=False, debug=False,
                   num_devices=N_CORES)

    x_in = nc.dram_tensor("x_pre", [128, KC, S], BF16, kind="ExternalInput")
    wq_in = nc.dram_tensor("wq_pre", [HPC, 128, KC, 128], BF16, kind="ExternalInput")
    wk_in = nc.dram_tensor("wk_pre", [HPC, 128, KC, 128], BF16, kind="ExternalInput")
    wv_in = nc.dram_tensor("wv_pre", [128, KC, 2, 512], BF16, kind="ExternalInput")
    wo_in = nc.dram_tensor("wo_pre", [128, KC, 4, 512], BF16, kind="ExternalInput")
    bq_in = nc.dram_tensor("bq_p", [128, HPC, 1], F32, kind="ExternalInput")
    bk_in = nc.dram_tensor("bk_p", [128, HPC, 1], F32, kind="ExternalInput")
    bv_in = nc.dram_tensor("bv_p", [128, 2, 512], F32, kind="ExternalInput")
    csk_in = nc.dram_tensor("csk2", [128, S], BF16, kind="ExternalInput")
    ssk_in = nc.dram_tensor("ssk2", [128, S], BF16, kind="ExternalInput")
    tri_in = nc.dram_tensor("tri01", [128, 128], BF16, kind="ExternalInput")
    ones_in = nc.dram_tensor("ones128", [128, 128], BF16, kind="ExternalInput")
    y_out = nc.dram_tensor("y", [NQ, DIM], F32, kind="ExternalOutput")

    with tile.TileContext(nc) as tc:
        with (
            tc.tile_pool(name="consts", bufs=1) as consts,
            tc.tile_pool(name="xpool", bufs=1) as xpool,
            tc.tile_pool(name="qkv", bufs=1) as qkv,
            tc.tile_pool(name="wqk", bufs=4) as wqk,
            tc.tile_pool(name="combp", bufs=4) as combp,
            tc.tile_pool(name="wbig", bufs=2) as wbig,
            tc.tile_pool(name="rope", bufs=3) as rope,
            tc.tile_pool(name="ptp", bufs=5) as ptp,
            tc.tile_pool(name="dram", bufs=1, space="DRAM") as dram,
            tc.tile_pool(name="psB", bufs=2, space="PSUM") as psB,
            tc.tile_pool(name="psL1", bufs=1, space="PSUM") as psL1,
            tc.tile_pool(name="psO1", bufs=1, space="PSUM") as psO1,
        ):
            # ---- DMAs in consumption order: first Q/K weights + x ----
            wq_sbs = {}
            wk_sbs = {}

            def fetch_w(which, h, nsplit=1):
                t = wqk.tile([128, KC, 128], BF16, tag="w",
                             name=f"w{which}_sb")
                src = (wq_in if which == "q" else wk_in)[h]
                step = KC // nsplit
                for j in range(0, KC, step):
                    nc.sync.dma_start(t[:, j:j + step, :], src[:, j:j + step, :])
                (wq_sbs if which == "q" else wk_sbs)[h] = t

            # x streams on the Act engine's DMA queue, weights on sync's,
            # so the two overlap from t=0. First tiles split fine so the
            # first matmul's semaphore wait covers minimal data.
            x_sb = xpool.tile([128, KC, S], BF16, tag="x", name="x_sb")
            fetch_w("q", 0, nsplit=4)
            nc.scalar.dma_start(x_sb[:, 0:1, :], x_in[:, 0:1, :])
            nc.scalar.dma_start(x_sb[:, 1:2, :], x_in[:, 1:2, :])
            fetch_w("k", 0, nsplit=2)
            nc.scalar.dma_start(x_sb[:, 2:4, :], x_in[:, 2:4, :])
            bq_sb = consts.tile([128, HPC, 1], F32, tag="bq")
            bk_sb = consts.tile([128, HPC, 1], F32, tag="bk")
            nc.sync.dma_start(bq_sb[:], bq_in[:])
            nc.sync.dma_start(bk_sb[:], bk_in[:])
            fetch_w("q", 1)
            fetch_w("k", 1)
            # second half of x rides the sync queue behind the first two
            # heads' weights, so both hardware-DGE queues fill x in
            # parallel and the projection never starves.
            for g in range(2, 4):
                nc.scalar.dma_start(x_sb[:, g * 2:g * 2 + 2, :],
                                    x_in[:, g * 2:g * 2 + 2, :])
            for g in range(4, 8):
                nc.sync.dma_start(x_sb[:, g * 2:g * 2 + 2, :],
                                  x_in[:, g * 2:g * 2 + 2, :])
            csk_sb = consts.tile([128, S], BF16, tag="cs", name="csk_sb")
            ssk_sb = consts.tile([128, S], BF16, tag="ss", name="ssk_sb")
            nc.sync.dma_start(csk_sb[:], csk_in[:])
            nc.sync.dma_start(ssk_sb[:], ssk_in[:])
            ones_sb = consts.tile([128, 128], BF16)
            nc.sync.dma_start(ones_sb[:], ones_in[:])
            tri_sb = consts.tile([128, 128], BF16)
            nc.sync.dma_start(tri_sb[:], tri_in[:])
            bv_sb = consts.tile([128, 2, 512], F32, tag="bv")
            nc.sync.dma_start(bv_sb[:], bv_in[:])

            q_sb = qkv.tile([128, HPC, S], BF16, tag="q", name="q_sb")
            k_sb = qkv.tile([128, HPC, S], BF16, tag="k", name="k_sb")
            v_sb = qkv.tile([128, NKT, 2, 512], BF16, tag="v", name="v_sb")
            ot_sb = qkv.tile([128, HPC, S], BF16, tag="ot", name="ot_sb")

            def rope_block(pm, b_ap, dst_ap):
                """dst = rope(pm + bias); pm [128, S] psum."""
                tmp_t = rope.tile([128, S], BF16, tag="tmp", name="tmp_t")
                nc.scalar.activation(tmp_t[:], pm, Act.Identity, bias=b_ap)
                tswap_t = rope.tile([128, S], BF16, tag="tswap", name="tswap_t")
                nc.scalar.dma_start(tswap_t[0:64, :], tmp_t[64:128, :])
                nc.scalar.dma_start(tswap_t[64:128, :], tmp_t[0:64, :])
                nc.vector.tensor_mul(tmp_t[:], tmp_t[:], csk_sb[:])
                nc.vector.tensor_mul(tswap_t[:], tswap_t[:], ssk_sb[:])
                nc.vector.tensor_add(dst_ap, tmp_t[:], tswap_t[:])

            # ---- Q/K projections + RoPE (weights double-prefetched) ----
            HALVES = [slice(0, 512), slice(512, 1024)]
            for h in range(HPC):
                wq_sb = wq_sbs[h]
                pm = psB.tile([128, S], F32, tag="big", name="pmq")
                for kc in range(KC):
                    for hs in HALVES:
                        nc.tensor.matmul(pm[:, hs], wq_sb[:, kc, :],
                                         x_sb[:, kc, hs],
                                         start=(kc == 0), stop=(kc == KC - 1))
                rope_block(pm[:], bq_sb[:, h, :], q_sb[:, h, :])
                if h + 2 < HPC:
                    fetch_w("q", h + 2)
                wk_sb = wk_sbs[h]
                pm = psB.tile([128, S], F32, tag="big", name="pmk")
                for kc in range(KC):
                    for hs in HALVES:
                        nc.tensor.matmul(pm[:, hs], wk_sb[:, kc, :],
                                         x_sb[:, kc, hs],
                                         start=(kc == 0), stop=(kc == KC - 1))
                rope_block(pm[:], bk_sb[:, h, :], k_sb[:, h, :])
                if h + 2 < HPC:
                    fetch_w("k", h + 2)

            # ---- V projection (transposed: rows on partitions) ----
            for eb in range(2):
                wv_sb = wbig.tile([128, KC, 512], BF16, tag="wv", name="wv_sb")
                nc.sync.dma_start(wv_sb[:], wv_in[:, :, eb, :])
                for rc in range(NKT):
                    rcols = slice(rc * 128, rc * 128 + 128)
                    pm = psB.tile([128, S], F32, tag="big", name="pmv")
                    for kc in range(KC):
                        nc.tensor.matmul(pm[:, 0:512], x_sb[:, kc, rcols],
                                         wv_sb[:, kc, :],
                                         start=(kc == 0), stop=(kc == KC - 1))
                    nc.vector.tensor_add(v_sb[:, rc, eb, :], pm[:, 0:512],
                                         bv_sb[:, eb, :])

            # ---- pair-exchange plumbing ----
            pid = nc.sync.partition_id()
            myrh = pid % 2
            prh = (pid + 1) % 2
            # staggered gathers: heads 0-3, 4-5, 6, 7 (tail kept tiny so
            # the final gather lands before the O-projection needs it)
            AG_HEADS = [(0, 4), (4, 2), (6, 1), (7, 1)]
            cc_in = [dram.tile([128, nh, 512], BF16, name=f"cc_in{g}")
                     for g, (h0, nh) in enumerate(AG_HEADS)]
            cc_out = [dram.tile([2, 128, nh, 512], BF16, name=f"cc_out{g}")
                      for g, (h0, nh) in enumerate(AG_HEADS)]
            # comb quarters: O-projection d_in chunks, local-first order
            # (0-7 = my heads for my rows, 8-15 = partner heads; wo_pre is
            # packed per-core with the matching chunk order). Quarters so
            # each lands as soon as its heads/gather finish.
            combq = [combp.tile([128, 4, 512], BF16, tag="cq",
                                name=f"combq{j}") for j in range(4)]
            wo23 = []

            # ---- attention: full q-strips, depth-3 pipeline ----
            def vtile(h, kt):
                return v_sb[:, kt, h // 4, (h % 4) * 128:(h % 4) * 128 + 128]

            def segs(kt):
                lo = kt * 128
                if lo < 512:
                    return [slice(lo, 512), slice(512, 1024)]
                return [slice(lo, 1024)]

            head_acc = {}
            pending = []
            wo_store = {}

            def post_head(h):
                if h in (3, 5, 6, 7):
                    g = {3: 0, 5: 1, 6: 2, 7: 3}[h]
                    h0, nh = AG_HEADS[g]
                    nc.sync.dma_start(
                        cc_in[g][:],
                        ot_sb[:, h0:h0 + nh, ds(prh * 512, 512)])
                    nc.gpsimd.collective_compute(
                        "AllGather",
                        mybir.AluOpType.bypass,
                        replica_groups=GROUPS,
                        ins=[cc_in[g][:].opt()],
                        outs=[cc_out[g][:].opt()],
                    )
                    # partner slot -> comb chunks 8+ (static dst)
                    coff = HPC + sum(n for _, n in AG_HEADS[:g])
                    nc.sync.dma_start(
                        combq[coff // 4][:, coff % 4:coff % 4 + nh, :],
                        cc_out[g][ds(prh, 1)][0])
                # stream wo during attention: ob 0-1 into the dead x_sb
                # buffer, ob 2-3 into the wv pool
                if h == 3:
                    t = xpool.tile([128, KC, 2, 512], BF16, tag="x",
                                   name="wo01_sb")
                    nc.sync.dma_start(t[:], wo_in[:, :, 0:2, :])
                    wo_store["01"] = t
                elif h in (5, 6):
                    t = wbig.tile([128, KC, 512], BF16, tag="wv",
                                  name="wo_sb")
                    nc.sync.dma_start(t[:], wo_in[:, :, h - 3, :])
                    wo23.append(t)

            def emit_lo(h, kt, pt):
                l_ps, o_ps = head_acc[h]
                for sg in segs(kt):
                    nc.tensor.matmul(l_ps[:, sg], ones_sb[:], pt[:, sg],
                                     start=(kt == 0), stop=(kt == NKT - 1))
                for sg in segs(kt):
                    nc.tensor.matmul(o_ps[:, sg], vtile(h, kt), pt[:, sg],
                                     start=(kt == 0), stop=(kt == NKT - 1))
                if kt == NKT - 1:
                    finish_head(h)

            def finish_head(h):
                l_ps, o_ps = head_acc.pop(h)
                rl = consts.tile([128, S], F32, tag="rl", name="rl")
                nc.vector.reciprocal_approx_fast(rl[:], l_ps[:])
                nc.vector.tensor_mul(ot_sb[:, h, :], o_ps[:], rl[:])
                # my-rows slice of this head -> comb chunk h (static dst)
                nc.sync.dma_start(combq[h // 4][:, h % 4, :],
                                  ot_sb[:, h, ds(myrh * 512, 512)])
                post_head(h)

            for h in range(HPC):
                head_acc[h] = (psL1.tile([128, S], F32, tag="l", name="l_ps"),
                               psO1.tile([128, S], F32, tag="o", name="o_ps"))
                for kt in range(NKT):
                    lo = kt * 128
                    st = psB.tile([128, S], F32, tag="big", name="st_ps")
                    for sg in segs(kt):
                        nc.tensor.matmul(
                            st[:, sg], k_sb[:, h, kt * 128:kt * 128 + 128],
                            q_sb[:, h, sg], start=True, stop=True)
                    pt = ptp.tile([128, S], BF16, tag="pt", name="pt")
                    nc.scalar.activation(pt[:, lo:1024], st[:, lo:1024],
                                         Act.Exp, scale=SCALE)
                    # diagonal block: zero out q < k after exp
                    nc.vector.tensor_mul(pt[:, lo:lo + 128],
                                         pt[:, lo:lo + 128], tri_sb[:])
                    pending.append((h, kt, pt))
                    if len(pending) > 3:
                        emit_lo(*pending.pop(0))
            while pending:
                emit_lo(*pending.pop(0))
            wo01_sb = wo_store["01"]

            # ---- output projection for my 512 rows ----
            # process ob-pairs with all 8 PSUM banks open and the final
            # gather's chunk (15) deferred to the pair's end, so ~25us of
            # matmuls cover the last AllGather's latency + pair skew.
            def omm(pms2, ob, i, qc):
                qsl = slice(qc * 128, qc * 128 + 128)
                osl = slice((qc % 2) * 512, (qc % 2) * 512 + 512)
                nc.tensor.matmul(pms2[qc // 2][:, osl],
                                 combq[i // 4][:, i % 4, qsl],
                                 (wo01_sb[:, i, ob, :] if ob < 2
                                  else wo23[ob - 2][:, i, :]),
                                 start=(i == 0), stop=(i == KC - 1))

            def drain(pms2, ob, qc):
                qsl = slice(qc * 128, qc * 128 + 128)
                osl = slice((qc % 2) * 512, (qc % 2) * 512 + 512)
                y_sb = rope.tile([128, 512], F32, tag="tswap", name="y_sb")
                if qc % 2 == 0:
                    nc.vector.tensor_copy(y_sb[:], pms2[qc // 2][:, osl])
                    nc.sync.dma_start(y_out[qsl, ob * 512:ob * 512 + 512],
                                      y_sb[:])
                else:
                    nc.scalar.activation(y_sb[:], pms2[qc // 2][:, osl],
                                         Act.Identity)
                    nc.scalar.dma_start(
                        y_out[qsl, ob * 512:ob * 512 + 512], y_sb[:])

            for pair in range(2):
                obs = (2 * pair, 2 * pair + 1)
                pmss = {
                    obs[0]: [psB.tile([128, S], F32, tag="big", name="pmoA"),
                             psB.tile([128, S], F32, tag="big", name="pmoB")],
                    obs[1]: [psL1.tile([128, S], F32, tag="l", name="pmoC"),
                             psO1.tile([128, S], F32, tag="o", name="pmoD")],
                }
                for i in range(KC - 1):
                    for ob in obs:
                        for qc in range(4):
                            omm(pmss[ob], ob, i, qc)
                for ob in obs:
                    for qc in range(4):
                        omm(pmss[ob], ob, KC - 1, qc)
                for ob in obs:
                    for qc in range(4):
                        drain(pmss[ob], ob, qc)
    nc.compile()
    return nc


def _get_nc():
    if "nc" not in _cache:
        _cache["nc"] = _build_nc()
    return _cache["nc"]


def _evenodd(a):
    # permute within-head dim: even indices first, then odd (axis 0)
    return np.concatenate([a[0::2], a[1::2]], axis=0)


def kernel(**inputs):
    from concourse.bass_utils import run_bass_kernel_spmd

    trace = bool(inputs.pop("_trace", False))
    x = np.asarray(inputs["x"], np.float32)
    freqs_cos = np.asarray(inputs["freqs_cos"], np.float32)
    freqs_sin = np.asarray(inputs["freqs_sin"], np.float32)
    mask = np.asarray(inputs["mask"], np.float32)
    wq = np.asarray(inputs["wq"], np.float32)
    bq = np.asarray(inputs["bq"], np.float32)
    wk = np.asarray(inputs["wk"], np.float32)
    bk = np.asarray(inputs["bk"], np.float32)
    wv = np.asarray(inputs["wv"], np.float32)
    bv = np.asarray(inputs["bv"], np.float32)
    wo = np.asarray(inputs["wo"], np.float32)
    bo = np.asarray(inputs["bo"], np.float32)

    cosT = freqs_cos.T
    sinT = freqs_sin.T
    csk2 = np.vstack([cosT, cosT]).astype(bfloat16)
    ssk2 = np.vstack([-sinT, sinT]).astype(bfloat16)

    m2 = mask[0, 0]  # [S(q), S(k)] additive
    # multiplicative 0/1 within-block causal mask, [k, q] layout
    tri01 = (m2[:128, :128].T == 0.0).astype(np.float32).astype(bfloat16)
    ones128 = np.ones((128, 128), np.float32).astype(bfloat16)

    def pack_thin(w_half):
        out = np.empty((HPC, 128, KC, 128), bfloat16)
        for h in range(HPC):
            rows = _evenodd(w_half[h * 128:(h + 1) * 128])  # [128, 2048]
            out[h] = rows.T.reshape(KC, 128, 128).transpose(1, 0, 2).astype(bfloat16)
        return out

    halves = []
    for hh in range(2):
        sl = slice(hh * 1024, hh * 1024 + 1024)
        wq_pre = pack_thin(wq[sl])
        wk_pre = pack_thin(wk[sl])
        bq_p = np.empty((128, HPC, 1), np.float32)
        bk_p = np.empty((128, HPC, 1), np.float32)
        for h in range(HPC):
            bq_p[:, h, 0] = _evenodd(bq[hh * 1024 + h * 128:hh * 1024 + (h + 1) * 128])
            bk_p[:, h, 0] = _evenodd(bk[hh * 1024 + h * 128:hh * 1024 + (h + 1) * 128])
        wv_pre = np.ascontiguousarray(
            wv[sl].T.reshape(KC, 128, 2, 512).transpose(1, 0, 2, 3)
        ).astype(bfloat16)
        # wo d_in chunks rolled so local head chunks come first (matches
        # comb's local-first layout in the kernel)
        order = list(range(hh * 8, hh * 8 + 8)) + \
            list(range((1 - hh) * 8, (1 - hh) * 8 + 8))
        woT = wo.T.reshape(KC, 128, 4, 512)
        wo_pre = np.ascontiguousarray(
            woT[order].transpose(1, 0, 2, 3)
        ).astype(bfloat16)
        halves.append((wq_pre, wk_pre, bq_p, bk_p, wv_pre, wo_pre))

    common = {
        "csk2": csk2, "ssk2": ssk2,
        "tri01": tri01, "ones128": ones128,
    }
    in_maps = []
    for c in range(N_CORES):
        b, hh = c // 2, c % 2
        wq_pre, wk_pre, bq_p, bk_p, wv_pre, wo_pre = halves[hh]
        x_pre = np.ascontiguousarray(
            x[b].T.reshape(KC, 128, S).transpose(1, 0, 2)
        ).astype(bfloat16)
        bv_p = np.ascontiguousarray(
            np.broadcast_to(bv[hh * 1024:hh * 1024 + 1024].reshape(1, 2, 512),
                            (128, 2, 512))
        )
        in_maps.append({
            **common,
            "x_pre": x_pre,
            "wq_pre": wq_pre, "wk_pre": wk_pre,
            "bq_p": bq_p, "bk_p": bk_p,
            "wv_pre": wv_pre, "bv_p": bv_p, "wo_pre": wo_pre,
        })

    nc = _get_nc()
    kwargs = {}
    if trace:
        kwargs = {"trace": True, "trace_cores": list(range(N_CORES))}
    res = run_bass_kernel_spmd(nc, in_maps, core_ids=list(range(N_CORES)), **kwargs)
    _cache["last_result"] = res

    out = np.empty((B, S, DIM), np.float32)
    for c in range(N_CORES):
        b, hh = c // 2, c % 2
        out[b, hh * NQ:hh * NQ + NQ] = res.results[c]["y"] + bo[None, :]
    return out
